# revision 2
# baseline (speedup 1.0000x reference)
"""Trainium2 Bass kernel: 2-layer GRU (H=200) + fc/tanh head, teacher-forced inputs.

Architecture (per NeuronCore, data-parallel over batch, 16 batch rows/core):
  - Layout: "H-major" — hidden/gate dims on SBUF partitions, batch on the free dim.
  - Gate pre-activations gh = W_hh @ h + b_hh computed per step as 12 small
    matmuls (6 gate-chunks of 100 x 2 K-chunks of ~100); biases folded in via a
    constant ones-row appended to the hidden state (K=101 for chunk 0).
  - Input projections gx0 (from x) and gx1 (from h0) are computed as batched
    chunk-GEMMs (32 timesteps at a time, N=512) off the recurrence critical path.
  - h0 history lives in an SBUF ring (5 chunks) feeding the gx1 chunk-GEMM;
    layer-1 scan runs one chunk behind layer-0, interleaved cell-by-cell so all
    engines stay busy.
  - fc output (4 x 16 per step) accumulates into one PSUM bank per chunk; a
    single tanh over [4, 512] flushes it to SBUF and DMA to HBM.
"""

import numpy as np

import concourse.bacc as bacc
import concourse.mybir as mybir
import concourse.tile as tile
from concourse import bass_utils

F32 = mybir.dt.float32
F32R = mybir.dt.float32r
AF = mybir.ActivationFunctionType

B = 128          # full batch
T = 1024         # timesteps
H = 200          # hidden size
HC = 100         # hidden chunk (2 chunks per H)
G3 = 3 * H       # 600 gate rows
NG = 6           # gate chunks of HC
IN0 = 8          # layer-0 input size
OUT = 4          # fc output size
NCORES = 8
BC = B // NCORES  # 16 batch rows per core
CH = 32          # timesteps per chunk
RING = 5         # h0 history ring depth (chunks)


def _build_nc(t_steps=T, ch=CH, reps=1, lag=0, probe=None):
    nchunk = t_steps // ch
    nc = bacc.Bacc("TRN2", target_bir_lowering=False, debug=False)

    x9 = nc.dram_tensor("x9", (IN0 + 1, t_steps * BC), F32, kind="ExternalInput")
    w0 = nc.dram_tensor("w0", (IN0 + 1, G3), F32, kind="ExternalInput")
    whh0a = nc.dram_tensor("whh0a", (HC + 1, G3), F32, kind="ExternalInput")
    whh0b = nc.dram_tensor("whh0b", (HC, G3), F32, kind="ExternalInput")
    wih1a = nc.dram_tensor("wih1a", (HC + 1, G3), F32, kind="ExternalInput")
    wih1b = nc.dram_tensor("wih1b", (HC, G3), F32, kind="ExternalInput")
    whh1a = nc.dram_tensor("whh1a", (HC + 1, G3), F32, kind="ExternalInput")
    whh1b = nc.dram_tensor("whh1b", (HC, G3), F32, kind="ExternalInput")
    wfca = nc.dram_tensor("wfca", (HC + 1, OUT), F32, kind="ExternalInput")
    wfcb = nc.dram_tensor("wfcb", (HC, OUT), F32, kind="ExternalInput")
    yt = nc.dram_tensor("yt", (OUT, t_steps * BC), F32, kind="ExternalOutput")

    # PSUM/gx free-layout positions (16-wide units) for gate-chunk gc (0..5 =
    # r0,r1,z0,z1,n0,n1) of each layer; rz of both layers contiguous [0:8),
    # n of both layers contiguous [8:12); state layout [h0k0 h0k1 h1k0 h1k1].
    POS0 = (0, 1, 4, 5, 8, 9)
    POS1 = (2, 3, 6, 7, 10, 11)

    with tile.TileContext(nc) as tc:
        with (
            tc.tile_pool(name="persist", bufs=1) as persist,
            tc.tile_pool(name="x9p", bufs=2) as x9p,
            tc.tile_pool(name="gxp", bufs=2) as gxp_pool,
            tc.tile_pool(name="outp", bufs=2) as outp,
            tc.tile_pool(name="elt", bufs=3) as elt,
            tc.tile_pool(name="ps_gx0", bufs=2, space="PSUM") as ps_gx0,
            tc.tile_pool(name="ps_gx1", bufs=2, space="PSUM") as ps_gx1,
            tc.tile_pool(name="ps_pair", bufs=3, space="PSUM") as ps_pair,
            tc.tile_pool(name="ps_fc", bufs=1, space="PSUM") as ps_fc,
        ):
            # ---- persistent SBUF tiles ----
            w0sb = persist.tile([IN0 + 1, G3], F32, tag="w0sb")
            whh0a_s = persist.tile([HC + 1, G3], F32, tag="whh0a")
            whh0b_s = persist.tile([HC, G3], F32, tag="whh0b")
            wih1a_s = persist.tile([HC + 1, G3], F32, tag="wih1a")
            wih1b_s = persist.tile([HC, G3], F32, tag="wih1b")
            whh1a_s = persist.tile([HC + 1, G3], F32, tag="whh1a")
            whh1b_s = persist.tile([HC, G3], F32, tag="whh1b")
            wfca_s = persist.tile([HC + 1, OUT], F32, tag="wfca")
            wfcb_s = persist.tile([HC, OUT], F32, tag="wfcb")
            # state ring: [101, ring-chunk, round-in-chunk, (h0k0 h0k1 h1k0 h1k1)x16]
            sdt = F32R if probe == "f32r" else F32
            ring = persist.tile([HC + 1, RING, ch, 4 * BC], sdt, tag="ring")

            for dst, src in (
                (w0sb, w0), (whh0a_s, whh0a), (whh0b_s, whh0b),
                (wih1a_s, wih1a), (wih1b_s, wih1b), (whh1a_s, whh1a),
                (whh1b_s, whh1b), (wfca_s, wfca), (wfcb_s, wfcb),
            ):
                nc.sync.dma_start(dst[:], src[:])

            # rows 0:100 zero (initial h), row 100 ones (bias row); partition
            # base must be quadrant-aligned so set all 1.0 then zero 0:100.
            ring_f = ring.bitcast(F32) if probe == "f32r" else ring
            nc.gpsimd.memset(ring_f[:], 1.0)
            nc.gpsimd.memset(ring_f[0:HC], 0.0)

            if probe == "f32r":
                # recurrent-path weights rounded once to f32r
                whh0a_r = persist.tile([HC + 1, G3], F32R, tag="whh0ar")
                whh0b_r = persist.tile([HC, G3], F32R, tag="whh0br")
                whh1a_r = persist.tile([HC + 1, G3], F32R, tag="whh1ar")
                whh1b_r = persist.tile([HC, G3], F32R, tag="whh1br")
                wih1a_r = persist.tile([HC + 1, G3], F32R, tag="wih1ar")
                wih1b_r = persist.tile([HC, G3], F32R, tag="wih1br")
                wfca_r = persist.tile([HC + 1, OUT], F32R, tag="wfcar")
                wfcb_r = persist.tile([HC, OUT], F32R, tag="wfcbr")
                for dst, srct in ((whh0a_r, whh0a_s), (whh0b_r, whh0b_s),
                                  (whh1a_r, whh1a_s), (whh1b_r, whh1b_s),
                                  (wih1a_r, wih1a_s), (wih1b_r, wih1b_s),
                                  (wfca_r, wfca_s), (wfcb_r, wfcb_s)):
                    nc.vector.tensor_copy(dst[:], srct[:])
                whh0a_s, whh0b_s = whh0a_r, whh0b_r
                whh1a_s, whh1b_s = whh1a_r, whh1b_r
                wih1a_s, wih1b_s = wih1a_r, wih1b_r
                wfca_s, wfcb_s = wfca_r, wfcb_r

            gx_tiles = {}
            fc_tiles = {}

            def slot(r):
                c, j = divmod(r % (RING * ch), ch)
                return ring[:, c, j]  # AP [101, 64]

            def get_gxp(rb):
                if rb not in gx_tiles:
                    gx_tiles[rb] = gxp_pool.tile([HC, ch, 12, BC], F32,
                                                 tag="gxt", name="gxt")
                return gx_tiles[rb]

            def gx0_chunk(i):
                # layer-0 input projections for L0 steps of round-block i
                x9t = x9p.tile([IN0 + 1, ch * BC], F32, tag="x9t", name="x9t")
                nc.sync.dma_start(x9t[:], x9[:, i * ch * BC:(i + 1) * ch * BC])
                gxt = get_gxp(i)
                for gc in range(NG):
                    pq = ps_gx0.tile([HC, ch * BC], F32, tag="q0", name="q0")
                    nc.tensor.matmul(pq[:], w0sb[:, gc * HC:(gc + 1) * HC],
                                     x9t[:], start=True, stop=True)
                    nc.scalar.copy(gxt[:, :, POS0[gc], :], pq[:])

            def gx1_chunk(c):
                # layer-1 input projections from h0 chunk c -> consumed in
                # round-block c+1 (L1 lags L0 by one chunk)
                rc = ring[:, c % RING]  # [101, ch, 64]
                gxt = get_gxp(c + 1)
                for gc in range(NG):
                    pq = ps_gx1.tile([HC, ch * BC], F32, tag="q1", name="q1")
                    nc.tensor.matmul(pq[:], wih1a_s[:, gc * HC:(gc + 1) * HC],
                                     rc[0:HC + 1, :, 0:BC], start=True, stop=False)
                    nc.tensor.matmul(pq[:], wih1b_s[:, gc * HC:(gc + 1) * HC],
                                     rc[0:HC, :, BC:2 * BC], start=False, stop=True)
                    nc.vector.tensor_copy(gxt[:, :, POS1[gc], :], pq[:])

            def pair_round(r, l0, l1):
                rb, j = divmod(r, ch)
                prev = slot(r - 1 - lag)
                cur = slot(r)
                gsl = get_gxp(rb)[:, j]  # [100, 12, 16]
                pg = ps_pair.tile([HC, 12 * BC], F32, tag="pg", name="pg")
                if probe == "nomm":
                    l0 = l1 = False
                    nc.vector.tensor_copy(pg[:, 0:BC], gsl[:, 0, :])
                def l0_mm(gc):
                    o = pg[:, POS0[gc] * BC:(POS0[gc] + 1) * BC]
                    nc.tensor.matmul(o, whh0a_s[:, gc * HC:(gc + 1) * HC],
                                     prev[0:HC + 1, 0:BC],
                                     start=True, stop=False)
                    nc.tensor.matmul(o, whh0b_s[:, gc * HC:(gc + 1) * HC],
                                     prev[0:HC, BC:2 * BC],
                                     start=False, stop=True)

                def l1_mm(gc):
                    o = pg[:, POS1[gc] * BC:(POS1[gc] + 1) * BC]
                    nc.tensor.matmul(o, whh1a_s[:, gc * HC:(gc + 1) * HC],
                                     prev[0:HC + 1, 2 * BC:3 * BC],
                                     start=True, stop=False)
                    nc.tensor.matmul(o, whh1b_s[:, gc * HC:(gc + 1) * HC],
                                     prev[0:HC, 3 * BC:4 * BC],
                                     start=False, stop=True)

                if l0:
                    for gc in range(NG):
                        l0_mm(gc)
                if l1:
                    for gc in range(NG):
                        l1_mm(gc)
                if probe == "noelt":
                    nc.vector.tensor_copy(cur[0:HC, 0:BC], pg[:, 0:BC])
                    return
                # merged elementwise over both layers (inactive half computes
                # bounded garbage that is never consumed)
                s = elt.tile([HC, 8 * BC], F32, tag="s", name="s")
                nc.vector.tensor_add(s[:], pg[:, 0:8 * BC], gsl[:, 0:8, :])
                rz = elt.tile([HC, 8 * BC], F32, tag="rz", name="rz")
                nc.scalar.activation(rz[:], s[:], AF.Sigmoid)
                tn = elt.tile([HC, 4 * BC], F32, tag="tn", name="tn")
                nc.vector.tensor_mul(tn[:], rz[:, 0:4 * BC], pg[:, 8 * BC:12 * BC])
                np_ = elt.tile([HC, 4 * BC], F32, tag="np", name="np")
                nc.vector.tensor_add(np_[:], tn[:], gsl[:, 8:12, :])
                n_ = elt.tile([HC, 4 * BC], F32, tag="n", name="n")
                nc.scalar.activation(n_[:], np_[:], AF.Tanh)
                d = elt.tile([HC, 4 * BC], F32, tag="d", name="d")
                nc.vector.tensor_sub(d[:], prev[0:HC, 0:4 * BC], n_[:])
                e = elt.tile([HC, 4 * BC], F32, tag="e", name="e")
                nc.vector.tensor_mul(e[:], rz[:, 4 * BC:8 * BC], d[:])
                nc.vector.tensor_add(cur[0:HC, 0:4 * BC], e[:], n_[:])

            def fc_flush(rb):
                # rounds [rb*ch, rb*ch+ch) carried L1 steps [(rb-1)*ch, rb*ch):
                # h1 of those steps sits in ring chunk rb%RING h1-halves.
                if probe == "nomm":
                    return
                rc = ring[:, rb % RING]  # [101, ch, 64]
                fcp = ps_fc.tile([OUT, ch * BC], F32, tag="fc", name="fct")
                nc.tensor.matmul(fcp[:], wfca_s[:], rc[0:HC + 1, :, 2 * BC:3 * BC],
                                 start=True, stop=False)
                nc.tensor.matmul(fcp[:], wfcb_s[:], rc[0:HC, :, 3 * BC:4 * BC],
                                 start=False, stop=True)
                ot = outp.tile([OUT, ch * BC], F32, tag="ot", name="ot")
                nc.scalar.activation(ot[:], fcp[:], AF.Tanh)
                nc.sync.dma_start(
                    yt[:, (rb - 1) * ch * BC:rb * ch * BC], ot[:])

            # ---- main pipelined loop over round-blocks ----
            for _rep in range(reps):  # reps>1 only for timing probes
                gx_tiles.clear()
                fc_tiles.clear()
                gx0_chunk(0)
                for rb in range(nchunk + 1):
                    l0 = rb < nchunk
                    l1 = rb >= 1
                    if l1:
                        gx1_chunk(rb - 1)
                        if rb == nchunk:
                            get_gxp(rb)  # tail block: no gx0 half
                    for j in range(ch):
                        pair_round(rb * ch + j, l0, l1)
                    if l1:
                        fc_flush(rb)
                    if rb == 0:
                        # L1 reads h1(-1)=0 from slot ch-1: head rounds wrote
                        # garbage into the h1 half; re-zero it.
                        c0, j0 = divmod(ch - 1, ch)
                        nc.gpsimd.memset(
                            ring_f[0:HC, c0, j0, 2 * BC:4 * BC], 0.0)
                    if l0 and rb + 1 < nchunk:
                        gx0_chunk(rb + 1)

    nc.compile()
    return nc


_NC_CACHE = {}


def _get_nc(t_steps=T, ch=CH, reps=1, lag=0, probe=None):
    key = (t_steps, ch, reps, lag, probe)
    if key not in _NC_CACHE:
        _NC_CACHE[key] = _build_nc(t_steps, ch, reps, lag, probe)
    return _NC_CACHE[key]


_RUNNER_CACHE = {}


def _get_runner(t_steps=T, ch=CH, reps=1, lag=0, probe=None):
    """Build (once) a cached jit'd SPMD executable for the compiled Bass module.

    Mirrors concourse.bass2jax.run_bass_via_pjrt but caches the jitted
    callable so repeated invocations don't retrace/recompile.
    """
    key = (t_steps, ch, reps, lag, probe)
    if key in _RUNNER_CACHE:
        return _RUNNER_CACHE[key]

    import jax
    from jax.sharding import Mesh, PartitionSpec
    from jax.experimental.shard_map import shard_map
    from concourse import bass2jax
    import concourse.mybir as _mybir

    nc = _get_nc(t_steps, ch, reps, lag, probe)
    bass2jax.install_neuronx_cc_hook()
    assert nc.dbg_addr is None
    pid_name = nc.partition_id_tensor.name if nc.partition_id_tensor else None

    in_names, out_names, out_avals = [], [], []
    for alloc in nc.m.functions[0].allocations:
        if not isinstance(alloc, _mybir.MemoryLocationSet):
            continue
        name = alloc.memorylocations[0].name
        if alloc.kind == "ExternalInput":
            if name != pid_name:
                in_names.append(name)
        elif alloc.kind == "ExternalOutput":
            out_names.append(name)
            out_avals.append(jax.core.ShapedArray(
                tuple(alloc.tensor_shape), _mybir.dt.np(alloc.dtype)))
    n_params = len(in_names)
    all_names = in_names + out_names
    if pid_name is not None:
        all_names = all_names + [pid_name]
    donate = tuple(range(n_params, n_params + len(out_names)))

    def _body(*args):
        operands = list(args)
        if pid_name is not None:
            operands.append(bass2jax.partition_id_tensor())
        outs = bass2jax._bass_exec_p.bind(
            *operands,
            out_avals=tuple(out_avals),
            in_names=tuple(all_names),
            out_names=tuple(out_names),
            lowering_input_output_aliases=(),
            sim_require_finite=True,
            sim_require_nnan=True,
            nc=nc,
        )
        return tuple(outs)

    devices = jax.devices()[:NCORES]
    mesh = Mesh(np.asarray(devices), ("core",))
    in_specs = (PartitionSpec("core"),) * (n_params + len(out_names))
    out_specs = (PartitionSpec("core"),) * len(out_names)
    sharded = jax.jit(
        shard_map(_body, mesh=mesh, in_specs=in_specs, out_specs=out_specs,
                  check_rep=False),
        donate_argnums=donate, keep_unused=True)
    runner = (sharded, in_names, out_names, out_avals)
    _RUNNER_CACHE[key] = runner
    return runner


_DEV_IN_CACHE = {}
_OUT_POOL = {}


def _sharding():
    import jax
    from jax.sharding import Mesh, NamedSharding, PartitionSpec
    devices = jax.devices()[:NCORES]
    mesh = Mesh(np.asarray(devices), ("core",))
    return NamedSharding(mesh, PartitionSpec("core"))


def _device_inputs(in_maps, key, in_names):
    """Concat per-core inputs and stage them on device once; reuse across
    calls with the same (unmutated) host arrays."""
    import jax
    dev_key = (key, tuple(id(m[name]) for m in in_maps for name in in_names))
    hit = _DEV_IN_CACHE.get(key)
    if hit is not None and hit[0] == dev_key:
        return hit[1]
    concat_in = [np.concatenate([m[name] for m in in_maps], axis=0)
                 for name in in_names]
    shard = _sharding()
    dev_in = [jax.device_put(a, shard) for a in concat_in]
    dev_in = [a.block_until_ready() for a in dev_in]
    _DEV_IN_CACHE[key] = (dev_key, dev_in)
    _OUT_POOL.pop(key, None)
    return dev_in


def _exec_device(in_maps, t_steps=T, ch=CH, reps=1, lag=0, probe=None):
    """Run the cached executable on 8 cores; returns on-device output arrays
    (blocked until ready). Inputs are staged on device once and reused; the
    donated output buffers are recycled from the previous call's outputs
    (every output element is rewritten by the kernel)."""
    import jax
    key = (t_steps, ch, reps, lag, probe)
    sharded, in_names, out_names, out_avals = _get_runner(t_steps, ch, reps, lag, probe)
    dev_in = _device_inputs(in_maps, key, in_names)
    donate = _OUT_POOL.pop(key, None)
    if donate is None:
        shard = _sharding()
        donate = [jax.device_put(
            np.zeros((NCORES * a.shape[0], *a.shape[1:]), a.dtype), shard)
            for a in out_avals]
    out_arrs = sharded(*dev_in, *donate)
    out_arrs = jax.block_until_ready(out_arrs)
    _OUT_POOL[key] = list(out_arrs)
    return out_arrs, out_names, out_avals


def _exec(in_maps, t_steps=T, ch=CH, reps=1, lag=0, probe=None):
    """Run the cached executable on 8 cores; returns list of per-core out dicts."""
    out_arrs, out_names, out_avals = _exec_device(
        in_maps, t_steps, ch, reps, lag, probe)
    out_np = [np.asarray(o) for o in out_arrs]
    return [
        {name: out_np[i].reshape(NCORES, *out_avals[i].shape)[c]
         for i, name in enumerate(out_names)}
        for c in range(NCORES)
    ]


def _prep_weights(W_ih0, W_hh0, b_ih0, b_hh0, W_ih1, W_hh1, b_ih1, b_hh1,
                  W_fc, b_fc):
    f = lambda a: np.ascontiguousarray(np.asarray(a, np.float32))
    W_ih0, W_hh0, W_ih1, W_hh1, W_fc = map(f, (W_ih0, W_hh0, W_ih1, W_hh1, W_fc))
    b_ih0, b_hh0, b_ih1, b_hh1, b_fc = map(f, (b_ih0, b_hh0, b_ih1, b_hh1, b_fc))
    cat = lambda w, bias: np.ascontiguousarray(
        np.concatenate([w[:, :HC].T, bias[None, :]], axis=0), np.float32)
    return {
        "w0": np.ascontiguousarray(
            np.concatenate([W_ih0.T, b_ih0[None, :]], axis=0), np.float32),
        "whh0a": cat(W_hh0, b_hh0),
        "whh0b": np.ascontiguousarray(W_hh0[:, HC:].T),
        "wih1a": cat(W_ih1, b_ih1),
        "wih1b": np.ascontiguousarray(W_ih1[:, HC:].T),
        "whh1a": cat(W_hh1, b_hh1),
        "whh1b": np.ascontiguousarray(W_hh1[:, HC:].T),
        "wfca": cat(W_fc, b_fc),
        "wfcb": np.ascontiguousarray(W_fc[:, HC:].T),
    }


def _make_in_maps(x, weights, t_steps=T):
    bsz = x.shape[0]
    emotion = x[:, 0, 4:8]
    tf = np.concatenate([np.ones((bsz, 1, 4), np.float32), x[:, :-1, 0:4]], axis=1)
    inputs = np.concatenate(
        [tf, np.broadcast_to(emotion[:, None, :], (bsz, t_steps, 4))], axis=-1)

    in_maps = []
    for c in range(NCORES):
        xs = inputs[c * BC:(c + 1) * BC]  # [16, t, 8]
        x9 = np.empty((IN0 + 1, t_steps * BC), np.float32)
        x9[0:IN0] = xs.transpose(2, 1, 0).reshape(IN0, t_steps * BC)
        x9[IN0] = 1.0
        m = dict(weights)
        m["x9"] = x9
        in_maps.append(m)
    return in_maps


def _run(x, weights, t_steps=T, ch=CH):
    """x: (B, t_steps, 8) float32 teacher-forcing raw input (as in reference)."""
    in_maps = _make_in_maps(x, weights, t_steps)
    results = _exec(in_maps, t_steps, ch)
    outs = [np.transpose(r["yt"].reshape(OUT, t_steps, BC), (2, 1, 0))
            for r in results]
    return np.concatenate(outs, axis=0), results


def kernel(x, W_ih0, W_hh0, b_ih0, b_hh0, W_ih1, W_hh1, b_ih1, b_hh1,
           W_fc, b_fc, xlens):
    x = np.ascontiguousarray(np.asarray(x, np.float32))
    weights = _prep_weights(W_ih0, W_hh0, b_ih0, b_hh0, W_ih1, W_hh1,
                            b_ih1, b_hh1, W_fc, b_fc)
    out, _ = _run(x, weights, T, CH)
    return out

